# revision 96
# baseline (speedup 1.0000x reference)
"""Tied-row (MSA) attention on 8 Trainium2 NeuronCores.

Reference computation (B=128, n=512, dim=256, h=8, dh=64,
r=tie_attn_dim=64, b=B//r=2):
    q = x @ Wq ; k,v = split(x @ Wkv)
    dots[b,h,i,j] = sum_{r,d} q[b,r,h,i,d] k[b,r,h,j,d] * scale
    attn = softmax_j(dots)
    out[b,r,h,i,d] = sum_j attn[b,h,i,j] v[b,r,h,j,d]
    y = out @ Wo + bo

Sharding: 8 cores = b(2) x head-pairs(4).  Each core owns one batch
element and 2 of the 8 heads and produces the partial
    y_part = out[:, :, own 2 heads, :] @ Wo[own 128 rows, :]
in f16; the host sums the 4 partials per b in f32 and adds bo.

Per-core device kernel (shapes hardcoded):
  inputs : xT [64, 256, 512] f16   (x[b] transposed to [r, c, n])
           wq,wk,wv [128, 2, 128] f16 (host-prearranged [p, cc, hp];
           wq pre-scaled by dh^-.5 * r^-.5)
           wo [128, 256] f16
  output : y  [64, 512, 256] f16   (partial)

All reductions feed the PE with full K=128 contraction chunks (the
cost dimension is the moving-free size only, so half-height K=64
matmuls waste PE):
  - dots contracts (r, d) in chunks of 128 by pairing consecutive MSA
    rows on the partition axis: qk2 [(r%2)*64+d, qk, h, rr, n] f16, so
    dots is 2h x 4i x 32rr matmuls of F=512 (vs 64 r-steps of K=64).
  - out[(r%2)*64+d, i] per (h, rr) uses v2 [j, jc, h, (r%2)*64+d] as
    stationary and attnT[h] [j, jc, i] as moving: 2h x 32rr x 4jc
    matmuls of F=512 with all 128 output partitions live.
  - y per r needs out in [hd, i] layout: 64-partition interleave
    copies out_ps[h][p*64:...] -> out_sb[h*64:...] recover K=128.

Cost-model facts that shape the schedule: a matmul costs its moving-
free size only (K and partitions are free, LDWEIGHTS ~free); an
engine op costs its free size (partitions free) plus ~0.2us fixed;
all DMA shares one serial ~344 GB/s device, each transfer needs a
~650ns queue config + ~900ns completion sem; the PE p-state ramps
over the first ~3us and resets on gaps.  Two Tile-framework rules:
cross-engine READERS of one pool tile serialize (the last reader is
the releaser and waits the others), and the dep tracker is interval-
based, so two engines writing interleaved byte ranges of a tile get
a false WAW — keep each engine's writes a contiguous block.

Schedule (PE-bound, ~218.5us of PE work, sim ~232.9us):
  Phase 1 streams x once on SP (weights host-prearranged to SBUF
  layout so their transfers are small and early), computes q/k per
  row into separate 1-bank PSUM tiles (tag bufs=4 so the cross-
  engine release chains fit the rotation budget), and interleaves
  straight from PSUM with four 64-partition copies (DVE h0, ACT h1;
  q copies start while the k matmuls run).  Every 12th row instead
  stages to SBUF (one copy per engine) and lets the idle GPSIMD do
  the fold, holding DVE/ACT under the 1.28us/row PE rate; rows 0-1
  borrow the dots banks so the fast early rows (no dots yet) never
  stall on the copy-release rotation.  dots i-tiles 0,1 accumulate
  one rr-step per pair, two rows behind the copies (capped in-loop;
  the rest run densely in the phase tail).
  Wave-two dots tiles rotate into the RELEASED qk banks (same tag):
  the first tile's 31 leading rr-steps keep the PE busy while the
  last pair's copies land, and no wave-two tile ever waits on a
  wave-one softmax.  softmax uses reciprocal_approx_fast (~5x
  cheaper than InstReciprocal, plenty for a softmax denominator).
  Phase 3 reloads x (v proj) and runs B/Ccopies/A/Cmm per r-pair
  with the A pairs 3 ahead and the y matmuls 2 pairs BEHIND their
  out matmuls, so the out_sb copy chains (~3.5us worst) are never
  on the PE's critical path.  out mms h-major; o[0]'s two half-
  copies on DVE, o[1]'s on ACT (same-engine readers avoid the
  release serialization).  Per pair DVE/ACT carry ~3.2/2.8us of
  copies against the 3.4us PE period.  y rides the Pool SWDGE
  queue per-row so the serial DMA device drains as rows complete;
  the last two rows go half-row on SP and the tail is one 364ns
  transfer.

Built with bacc.Bacc(): its compile() pass legalizes Tile's sync for
walrus (which caps sync waits per instruction); callers must
finalize() the program before running (see _get_program).
"""

import os
import sys

for _p in ("/opt/trn_rl_repo", "/root/.axon_site/_ro/trn_rl_repo"):
    if os.path.isdir(_p) and _p not in sys.path:
        sys.path.insert(0, _p)

import numpy as np

R = 64          # tie dim (MSA rows per batch element)
RR = 32         # r-pairs
XB = 4          # rows per x DMA block
RBY = 4         # rows per y DMA block
N = 512         # sequence length
C = 256         # model dim
HP = 128        # head-pair width: 2 heads x 64
E = 256         # output dim
NCORES = 8

_CACHE = {}


def build_program(phases=(1, 2, 3)):
    import concourse.bacc as bacc
    from concourse import mybir
    from concourse.tile import TileContext
    from contextlib import ExitStack

    f32 = mybir.dt.float32
    f16 = mybir.dt.float16

    nc = bacc.Bacc()
    xT = nc.declare_dram_parameter("xT", [R, C, N], f16, isOutput=False)
    # wq/wk/wv are pre-arranged on the host to the SBUF layout
    # [p, cc, hp] so their loads are one contiguous descriptor per
    # partition (the on-device (cc p)->(p cc) rearrange costs ~2x the
    # bytes in DMA-device time and sits in front of the first x row)
    wq = nc.declare_dram_parameter("wq", [128, 2, HP], f16, isOutput=False)
    wk = nc.declare_dram_parameter("wk", [128, 2, HP], f16, isOutput=False)
    wv = nc.declare_dram_parameter("wv", [128, 2, HP], f16, isOutput=False)
    wo = nc.declare_dram_parameter("wo", [HP, E], f16, isOutput=False)
    y = nc.declare_dram_parameter("y", [R, N, E], f16, isOutput=True)

    xT_blk = xT.rearrange("(rb r) (cc p) n -> rb p r cc n", r=XB, p=128)
    y_blk = y.rearrange("(rb r) (t p) e -> rb p r t e", r=RBY, p=128)

    def copy_eng(e, out, in_):
        if e % 2 == 0:
            nc.vector.tensor_copy(out, in_)
        else:
            nc.scalar.copy(out, in_)

    with TileContext(nc) as tc, ExitStack() as ctx:
        singles = ctx.enter_context(tc.tile_pool(name="singles", bufs=1))
        sm = ctx.enter_context(tc.tile_pool(name="sm", bufs=4))
        attnp = ctx.enter_context(tc.tile_pool(name="attnp", bufs=4))
        attntp = ctx.enter_context(tc.tile_pool(name="attntp", bufs=2))
        xpool = ctx.enter_context(tc.tile_pool(name="xpool", bufs=3))
        vpool = ctx.enter_context(tc.tile_pool(name="vpool", bufs=10))

        WARM = 3            # phase-3 A-pair warmup depth
        x_tiles = {}
        v2s = {}

        def stage_a(r, ps, tag, bufs):
            """v projection + v2 staging for row r, v_ps from pool `ps`."""
            rb, ri = divmod(r, XB)
            if ri == 0:
                x_tiles[rb] = xpool.tile([128, XB, 2, N], f16,
                                         tag="x", name=f"x3_{rb}")
                nc.sync.dma_start(out=x_tiles[rb], in_=xT_blk[rb])
            x_sb = x_tiles[rb]
            p, rr = r % 2, r // 2
            v_ps = ps.tile([128, 4, 2, 64], f32, tag=tag, bufs=bufs,
                           name=f"v_ps_{r}")
            for jt in range(4):
                for cc in range(2):
                    nc.tensor.matmul(
                        v_ps[:, jt],
                        lhsT=x_sb[:, ri, cc, jt * 128:(jt + 1) * 128],
                        rhs=wv_sb[:, cc, :],
                        start=(cc == 0), stop=(cc == 1))
            if p == 0:
                # (p*64+d) must stay contiguous in the last dim: the out
                # matmul's stationary AP allows only ONE free dimension
                v2s[rr] = vpool.tile([128, 4, 2, 128], f16, tag="v2",
                                     name=f"v2_{rr}")
            # both heads in one multi-dim-AP copy: src free (jt, h, d),
            # dst free (jc, h, d) with the parity offset on d
            copy_eng(r, v2s[rr][:, :, :, p * 64:(p + 1) * 64], v_ps)

        # weights: [256, X] -> sbuf [128, 2, X] (c-chunk on free axis)
        wq_sb = singles.tile([128, 2, HP], f16)
        wk_sb = singles.tile([128, 2, HP], f16)
        wv_sb = singles.tile([128, 2, HP], f16)
        wo_sb = singles.tile([128, E], f16)

        # attnT survives into phase 3: kernel-scoped pool
        attnT = [attntp.tile([128, 4, N], f16, tag="attnT", name=f"attnT_{h}")
                 for h in range(2)]

        def softmax(dots_hit, h, it, mul_eng=None):
            """dots PSUM tile -> normalized f16 attn SBUF tile.

            No max-subtraction: dots = q k^T with the 1/(sqrt(dh) sqrt(r))
            scale folded into Wq, so entries are ~N(0,1) and exp cannot
            overflow fp32/fp16.  mul_eng picks the normalize engine: Pool
            (901ns, default) while DVE is loaded, DVE (~330ns, f16 2x
            mode) for the wave-two tiles whose attnT gates phase 3."""
            ssum = sm.tile([128, 1], f32, tag="ssum", bufs=8)
            rinv = sm.tile([128, 1], f32, tag="rinv", bufs=8)
            attn = attnp.tile([128, N], f16, tag="attn", bufs=8,
                              name=f"attn_{h}_{it}")
            nc.scalar.activation(
                out=attn, in_=dots_hit,
                func=mybir.ActivationFunctionType.Exp,
                accum_out=ssum)
            nc.vector.reciprocal_approx_fast(rinv, ssum)
            (mul_eng or nc.gpsimd).tensor_scalar_mul(attn, attn, rinv)
            return attn

        def transpose_attn(attn, h, it):
            # one f16 xbar DMA transpose, SBUF -> SBUF: out[j, jc, i] =
            # attn[i, jc*128 + j]; no PE/PSUM involvement
            nc.sync.dma_start_transpose(
                out=attnT[h][:, :, it * 128:(it + 1) * 128], in_=attn)

        # resident interleaved q/k, one tile: [(r%2)*64+d, h, rr, qk, n] f16.
        # h is the OUTERMOST free dim so the per-row DVE (h0) and ACT (h1)
        # interleave copies write disjoint byte intervals: the tile dep
        # tracker is interval-based, and interleaved writes would create a
        # false WAW that serializes the two engines.
        with tc.tile_pool(name="resid", bufs=1) as resid, \
             tc.tile_pool(name="stgp", bufs=3) as stgp:
            qk2 = resid.tile([128, 2, RR, 2, N], f16)
            # every 8th row's fold runs on the otherwise-idle GPSIMD via an
            # SBUF staging tile (Pool cannot read PSUM): DVE/ACT then carry
            # one 512-elem staging copy instead of two folds on those rows,
            # dropping their per-row load below the 1278ns PE rate
            POOLROWS = frozenset(range(2, R, 12))
            DOTS_CAP = 25

            def dots_mm(tile, h, ic, rr):
                nc.tensor.matmul(
                    tile,
                    lhsT=qk2[:, h, rr, 0, ic * 128:(ic + 1) * 128],
                    rhs=qk2[:, h, rr, 1, :],
                    start=(rr == 0), stop=(rr == RR - 1))

            # -------- Phase 1 + dots i-tiles 0,1 fused --------
            attn01 = {}
            with tc.tile_pool(name="ps1", space="PSUM", bufs=2) as ps1:
                dots01 = None   # allocated after row 1 (see below)
                n_r = R if 1 in phases else 0
                next_rr = 0

                def emit_dots_up_to(limit, budget):
                    nonlocal next_rr
                    while next_rr < min(limit, RR) and budget > 0:
                        for h in range(2):
                            for ic in (0, 1):
                                dots_mm(dots01[h][ic], h, ic, next_rr)
                        next_rr += 1
                        budget -= 1

                for r in range(n_r):
                    if r == 0:
                        # startup: first x row + wq land in parallel on
                        # separate queues (SP / ACT HWDGE / Pool-SWDGE) so
                        # the first projection starts ~3us in
                        x_sb = xpool.tile([128, XB, 2, N], f16, tag="x",
                                          name="x1_0")
                        # one FIFO queue (SP), transfers ordered by first
                        # use so the serial DMA device never makes the PE
                        # wait on a tensor it does not need yet; wv/wo are
                        # issued rows later (phase-3 tensors) so their
                        # transfers never sit in front of an x row
                        nc.sync.dma_start(out=wq_sb, in_=wq[:, :, :])
                        nc.sync.dma_start(out=x_sb[:, 0:1],
                                          in_=xT_blk[0, :, 0:1])
                        nc.sync.dma_start(out=wk_sb, in_=wk[:, :, :])
                        nc.sync.dma_start(out=x_sb[:, 1:2], in_=xT_blk[0, :, 1:2])
                        nc.sync.dma_start(out=x_sb[:, 2:3], in_=xT_blk[0, :, 2:3])
                        nc.sync.dma_start(out=x_sb[:, 3:XB], in_=xT_blk[0, :, 3:XB])
                    if r == 4:
                        nc.gpsimd.dma_start(out=wv_sb, in_=wv[:, :, :])
                    if r == 8:
                        nc.sync.dma_start(out=wo_sb, in_=wo[:, :])
                    if r < n_r:
                        rb, ri = divmod(r, XB)
                        if ri == 0 and rb > 0:
                            x_sb = xpool.tile([128, XB, 2, N], f16, tag="x",
                                              name=f"x1_{rb}")
                            nc.sync.dma_start(out=x_sb, in_=xT_blk[rb])
                        # q and k in SEPARATE 1-bank PSUM tiles (tag bufs=4):
                        # each tile's release chain (DVE first reader, ACT
                        # second reader = releaser, serialized by the pool's
                        # release ordering) must finish within the 2-row
                        # rotation budget; a combined 2-bank qk tile at
                        # bufs=2 would miss it by ~150ns/row.  Rows 0-1
                        # borrow the dots banks (whose accumulators see no
                        # write until row >= 4) so rows 0-3 all get FRESH
                        # banks: the early rows run at 852ns/row (no dots
                        # yet), faster than the ~2.2us release chains, and
                        # would otherwise stall ~1.5us on the rotation.
                        qk_tag = "dots" if r < 2 else "qk"
                        q_ps = ps1.tile([128, N], f32, tag=qk_tag, bufs=4,
                                        name=f"q_ps_{r}")
                        k_ps = ps1.tile([128, N], f32, tag=qk_tag, bufs=4,
                                        name=f"k_ps_{r}")
                        if r == 2:
                            # dots01 accumulators take rows 0-1's slots;
                            # their first writes (~9us) land after those
                            # rows' releases (~7-8.4us)
                            dots01 = [[ps1.tile([128, N], f32, tag="dots",
                                                bufs=4,
                                                name=f"dotsA_{h}_{ic}")
                                       for ic in range(2)] for h in range(2)]
                        p, rr = r % 2, r // 2
                        ds = slice(p * 64, (p + 1) * 64)
                        pool_row = r in POOLROWS
                        if pool_row:
                            qs = stgp.tile([128, 2, N], f16, tag="stg",
                                           name=f"stg_{r}")
                        for cc in range(2):
                            nc.tensor.matmul(q_ps, lhsT=wq_sb[:, cc, :],
                                             rhs=x_sb[:, ri, cc, :],
                                             start=(cc == 0), stop=(cc == 1))
                        # interleave (h -> r%2) straight from PSUM with 64-
                        # partition engine copies (DVE h0, ACT h1); q copies
                        # start while the k matmuls run
                        if pool_row:
                            nc.vector.tensor_copy(qs[:, 0, :], q_ps)
                        else:
                            nc.vector.tensor_copy(qk2[ds, 0, rr, 0, :],
                                                  q_ps[0:64, :])
                            nc.scalar.copy(qk2[ds, 1, rr, 0, :],
                                           q_ps[64:128, :])
                        for cc in range(2):
                            nc.tensor.matmul(k_ps, lhsT=wk_sb[:, cc, :],
                                             rhs=x_sb[:, ri, cc, :],
                                             start=(cc == 0), stop=(cc == 1))
                        if pool_row:
                            nc.scalar.copy(qs[:, 1, :], k_ps)
                            for h in range(2):
                                for qk in range(2):
                                    nc.gpsimd.tensor_copy(
                                        qk2[ds, h, rr, qk, :],
                                        qs[h * 64:(h + 1) * 64, qk, :])
                        else:
                            nc.vector.tensor_copy(qk2[ds, 0, rr, 1, :],
                                                  k_ps[0:64, :])
                            nc.scalar.copy(qk2[ds, 1, rr, 1, :],
                                           k_ps[64:128, :])
                    if 2 in phases and 1 in phases and r >= 4:
                        # dots step rr is gated on pair rr's interleave
                        # copies; the extra row of lag covers the slower
                        # GPSIMD folds (~4.6us behind their row).  Capping
                        # the in-loop steps keeps the per-row PE demand
                        # under the DVE copy rate; the deferred steps run
                        # densely in the phase tail
                        emit_dots_up_to(min((r - 2) // 2, DOTS_CAP), 1)

                if 2 in phases and 1 in phases:
                    # wave-two tiles rotate into the RELEASED qk banks (same
                    # tag, same 2KB/part size), not the dots banks: the first
                    # tile's 31 leading rr-steps keep the PE busy while the
                    # last pair's interleave copies land, and no wave-two
                    # tile ever waits on a wave-one softmax.
                    def dots_tile(h, ic):
                        tile = ps1.tile([128, N], f32, tag="qk", bufs=4,
                                        name=f"dotsB_{h}_{ic}")
                        for rr in range(RR):
                            dots_mm(tile, h, ic, rr)
                        return tile

                    emit_dots_up_to(RR - 2, RR)  # deferred steps (ready)
                    w02 = dots_tile(0, 2)
                    emit_dots_up_to(RR, RR)     # dots01 steps 30, 31 (tail)
                    attn02 = softmax(w02, 0, 2)
                    for h in range(2):
                        for ic in (0, 1):
                            attn01[(h, ic)] = softmax(dots01[h][ic], h, ic)
                    transpose_attn(attn02, 0, 2)
                    for (h, ic), attn in attn01.items():
                        transpose_attn(attn, h, ic)
                    for h, ic in [(1, 2), (0, 3), (1, 3)]:
                        tile = dots_tile(h, ic)
                        attn = softmax(tile, h, ic, mul_eng=nc.vector)
                        transpose_attn(attn, h, ic)
        # ---------------- Phase 3: v, out, y (SW pipeline) ---------
        with tc.tile_pool(name="ps3", space="PSUM", bufs=2) as ps3, \
             tc.tile_pool(name="outp", bufs=12) as outp, \
             tc.tile_pool(name="ypool", bufs=4) as ypool:
            n_t = RR if 3 in phases else 0
            out_ps_s = {}
            out_sbs = {}
            y_sbs = {}
            if n_t:
                for r0 in range(2 * WARM):
                    # rows 2-3 borrow the (idle, same-sized) out-tag banks
                    # so the warmup never waits on the 2-buffer v rotation
                    stage_a(r0, ps3, "out" if r0 in (2, 3) else "v", 2)

            def stage_b(rr):
                o = [ps3.tile([128, N], f32, tag="out", bufs=2,
                              name=f"out_ps_{rr}_{h}") for h in range(2)]
                v2 = v2s.pop(rr)
                # h-major: o[0] stops 852ns early so its out_sb copies can
                # start while o[1] accumulates
                for h in range(2):
                    for jc in range(4):
                        nc.tensor.matmul(
                            o[h],
                            lhsT=v2[:, jc, h, :],
                            rhs=attnT[h][:, jc, :],
                            start=(jc == 0), stop=(jc == 3))
                out_ps_s[rr] = o

            def stage_c_copies(rr):
                # four 64-partition half-copies; BOTH readers of each o[h]
                # stay on one engine (cross-engine readers of a pool tile
                # serialize via the release ordering)
                o = out_ps_s[rr]
                sb0 = outp.tile([128, N], f16, tag="outsb",
                                name=f"out_sb_{2 * rr}")
                sb1 = outp.tile([128, N], f16, tag="outsb",
                                name=f"out_sb_{2 * rr + 1}")
                nc.vector.tensor_copy(sb0[0:64, :], o[0][0:64, :])
                nc.vector.tensor_copy(sb1[0:64, :], o[0][64:128, :])
                nc.scalar.copy(sb0[64:128, :], o[1][0:64, :])
                nc.scalar.copy(sb1[64:128, :], o[1][64:128, :])
                out_sbs[2 * rr] = sb0
                out_sbs[2 * rr + 1] = sb1

            def stage_c_mm(r):
                p, rr = r % 2, r // 2
                out_sb = out_sbs.pop(r)
                if p == 1:
                    out_ps_s.pop(rr)
                y_ps = ps3.tile([128, 4, E], f32, tag="y", bufs=2,
                                name=f"y_ps_{r}")
                for ic in range(4):
                    nc.tensor.matmul(
                        y_ps[:, ic, :],
                        lhsT=out_sb[:, ic * 128:(ic + 1) * 128],
                        rhs=wo_sb,
                        start=True, stop=True)
                yb, ryi = divmod(r, RBY)
                if ryi == 0:
                    y_sbs[yb] = ypool.tile([128, RBY, 4, E], f16, tag="ysb",
                                           name=f"y_sb_{yb}")
                if r >= R - RBY:
                    # drain path: per-row single-engine copies split in two
                    # region-pieces (first half starts after ic1's matmul).
                    # DMA configs cost ~650ns each on a queue's sequencer,
                    # so rows 60-61 ride the Pool SWDGE queue whole-row and
                    # only rows 62-63 go half-row on SP, leaving a single
                    # 364ns transfer as the program tail.
                    eng = (nc.vector.tensor_copy if r % 2 == 0
                           else nc.scalar.copy)
                    sp = 3 if r == R - 1 else 2
                    eng(y_sbs[yb][:, ryi, 0:sp, :], y_ps[:, 0:sp, :])
                    if r >= R - 2:
                        nc.sync.dma_start(out=y_blk[yb][:, ryi, 0:sp],
                                          in_=y_sbs[yb][:, ryi, 0:sp])
                    eng(y_sbs[yb][:, ryi, sp:4, :], y_ps[:, sp:4, :])
                    if r >= R - 2:
                        nc.sync.dma_start(out=y_blk[yb][:, ryi, sp:4],
                                          in_=y_sbs[yb][:, ryi, sp:4])
                    else:
                        nc.gpsimd.dma_start(out=y_blk[yb][:, ryi],
                                            in_=y_sbs[yb][:, ryi])
                    if r == R - 1:
                        y_sbs.pop(yb)
                else:
                    copy_eng(r, y_sbs[yb][:, ryi, :, :], y_ps)
                    # per-row writeback keeps the serial DMA device drained
                    # as rows complete instead of queueing 3us block
                    # transfers in front of the final rows at the tail
                    nc.gpsimd.dma_start(out=y_blk[yb][:, ryi],
                                        in_=y_sbs[yb][:, ryi])
                    if ryi == RBY - 1:
                        y_sbs.pop(yb)

            # warmup three A-pairs (fills the PE while the last wave-two
            # warmup A-pairs were emitted in the ps1 scope; per iteration:
            # B(t-1), Ccopies(t-1), A-pair(t+2), Cmm(t-2).  The y matmuls
            # run TWO pairs behind their out matmuls so the out_sb copy
            # chains (~3.6us worst) are never on the PE's critical path.
            for t in range(1, n_t + 3):
                if t - 1 < n_t:
                    stage_b(t - 1)
                    stage_c_copies(t - 1)
                for p_ in range(WARM, n_t):
                    # taper: the last three A-pairs run at lead 1 so the
                    # final iterations keep PE work while copies drain
                    t_ = p_ - 2 if p_ <= n_t - 4 else p_ - 1
                    if t_ == t:
                        stage_a(2 * p_, ps3, "v", 2)
                        stage_a(2 * p_ + 1, ps3, "v", 2)
                if 2 <= t < n_t + 2:
                    stage_c_mm(2 * (t - 2))
                    stage_c_mm(2 * (t - 2) + 1)

    return nc


def _get_program():
    if "nc" not in _CACHE:
        nc = build_program()
        nc.finalize()
        _CACHE["nc"] = nc
    return _CACHE["nc"]


def make_in_maps(x, Wq, Wkv, Wo):
    """Host-side sharding: core = bi*4 + hpi."""
    scale = (64.0 ** -0.5) * (64.0 ** -0.5)
    x = np.asarray(x, np.float32)
    Wq = np.asarray(Wq, np.float32) * np.float32(scale)
    Wkv = np.asarray(Wkv, np.float32)
    Wo = np.asarray(Wo, np.float32)
    b = x.shape[0] // R
    xT = np.ascontiguousarray(
        x.reshape(b, R, N, C).transpose(0, 1, 3, 2)).astype(np.float16)
    def warr(a):
        # [C, HP] -> [p, cc, HP]: the SBUF layout, so the device load is
        # one contiguous descriptor per partition
        return np.ascontiguousarray(
            a.reshape(2, 128, HP).transpose(1, 0, 2)).astype(np.float16)

    in_maps = []
    for core in range(NCORES):
        bi, hpi = divmod(core, 4)
        cols = slice(hpi * HP, (hpi + 1) * HP)
        in_maps.append({
            "xT": xT[bi],
            "wq": warr(Wq[:, cols]),
            "wk": warr(Wkv[:, cols]),
            "wv": warr(Wkv[:, 512 + hpi * HP: 512 + (hpi + 1) * HP]),
            "wo": np.ascontiguousarray(Wo[cols, :]).astype(np.float16),
        })
    return in_maps


def combine_outputs(ys, bo):
    """ys: list of 8 [R, N, E] f16 partials in core order; returns [B, n, dim]."""
    ys = [np.asarray(t, np.float32) for t in ys]
    y0 = ys[0] + ys[1] + ys[2] + ys[3]
    y1 = ys[4] + ys[5] + ys[6] + ys[7]
    yy = np.concatenate([y0, y1], axis=0).reshape(2 * R, N, E)
    return (yy + np.asarray(bo, np.float32)).astype(np.float32)


def kernel(x, Wq, Wkv, Wo, bo, tie_attn_dim):
    assert int(tie_attn_dim) == R, f"hardcoded for tie_attn_dim={R}"
    from concourse.bass_utils import run_bass_kernel_spmd

    nc = _get_program()
    in_maps = make_in_maps(x, Wq, Wkv, Wo)
    res = run_bass_kernel_spmd(nc, in_maps, list(range(NCORES)))
    ys = [np.asarray(res.results[c]["y"], np.float32) for c in range(NCORES)]
    return combine_outputs(ys, bo)
